# revision 34
# baseline (speedup 1.0000x reference)
"""Trainium2 Bass kernel for nn_Attention_88441966559243.

Attention with additive bias [B,N,N] and per-key bool mask, fp32 in/out.
  B=2, N=2048, QD=1024, HEADS=16, DIM_HEAD=64.

Sharding: 8 cores = (batch b = core//4) x (query slice q0 = (core%4)*512).
Each core computes out[b, q0:q0+512, :] completely on-device; the host gather
is a pure concatenation. No collectives.

v4 design (serial-critical-path focused: For_i has an all-engine barrier at
the back-edge, so per-iteration latency is what's measured):
  - bf16 compute path everywhere except the bias (fp32 until exp; EB=exp(
    biasT+mask) quantized bf16 AFTER exp). Numpy model: relmax 6e-3 (gate 2e-2).
  - No DRAM scratch at all: kT chunks stay in SBUF (kst tiles) and v' is
    SBUF-resident, so per-iteration DMA is just inputs+outputs (~17 MB).
  - Emission order pipelines the serial chain: [x/xq/bias/w DMAs] -> A1
    transposes -> B (bias transpose + EB exp on ACT, hidden under A4/A2 PE)
    -> A4 v' -> A2 qT -> then per head-pair: kT m-chunk (PE) interleaved
    with that pair's attention block, so kT production hides under the
    ACT-bound exp stream.  PSUM->SBUF copies balanced across ACT/DVE.
  - C: both subheads' sim chunks in one 2-bank [128,1024] PSUM tile; one
    wide ACT exp -> ef bf16; DVE 2x multiply by EB -> eT; av matmuls lag 4
    chunks.  Denominator via v' ones column + PE replication + DVE
    reciprocal; normalized straight into packed oPair tiles.
  - D: out = oPair @ Wo + bo (wo pinned in SBUF across iterations).
"""
import sys
for _p in ("/opt/trn_rl_repo", "/root/.axon_site/_ro/trn_rl_repo"):
    if _p not in sys.path:
        sys.path.insert(0, _p)

import numpy as np
import ml_dtypes

import concourse.bass as bass
import concourse.mybir as mybir
from concourse import bacc
from concourse.tile import TileContext
from concourse.masks import make_identity
from concourse.bass_utils import run_bass_kernel_spmd

F = 1024          # feature dim (QD == INNER)
NK = 2048         # keys (full sequence)
Q = 512           # queries per core
H = 16            # heads
D = 64            # head dim
DV = 65           # head dim + ones column
SCALE = D ** -0.5
MASK_NEG = -30000.0

FC = F // 128      # 8 feature chunks
KC = NK // 128     # 16 key chunks
NB = NK // 512     # 4 key 512-blocks

f32 = mybir.dt.float32
fr = mybir.dt.float32r
bt = mybir.dt.bfloat16
AF = mybir.ActivationFunctionType


def build_nc(niter: int = 1, STAGES: str = "ABCD"):
    nc = bacc.Bacc(None, target_bir_lowering=False)

    x_in = nc.dram_tensor("x_in", [NK, F], bt, kind="ExternalInput")
    xq_in = nc.dram_tensor("xq_in", [Q, F], bt, kind="ExternalInput")
    bias_in = nc.dram_tensor("bias_in", [Q, NK], f32, kind="ExternalInput")
    maskneg_in = nc.dram_tensor("maskneg_in", [128, KC], f32, kind="ExternalInput")
    wq_in = nc.dram_tensor("wq_in", [F, F], bt, kind="ExternalInput")
    wkv_in = nc.dram_tensor("wkv_in", [F, 2 * F], bt, kind="ExternalInput")
    wo_in = nc.dram_tensor("wo_in", [F, F], bt, kind="ExternalInput")
    bo_in = nc.dram_tensor("bo_in", [1, F], fr, kind="ExternalInput")
    out_t = nc.dram_tensor("out_t", [Q, F], f32, kind="ExternalOutput")

    with TileContext(nc) as tc:
        with (
            tc.tile_pool(name="const", bufs=1) as constp,
            tc.tile_pool(name="psAB", bufs=2, space="PSUM") as psAB,
            tc.tile_pool(name="psC", bufs=2, space="PSUM") as psCp,
            tc.tile_pool(name="psu", bufs=2, space="PSUM") as psUp,
        ):
            # ---- constants & pinned wo ----
            ident = constp.tile([128, 128], f32)
            make_identity(nc, ident)
            ident_b = constp.tile([128, 128], bt)
            nc.scalar.copy(ident_b[:, :], ident[:, :])
            ones_f = constp.tile([128, 128], f32)
            nc.vector.memset(ones_f[:, :], 1.0)
            ones_r = constp.tile([128, 128], fr)
            nc.scalar.copy(ones_r[:, :], ones_f[:, :])
            ones_b = constp.tile([128, 128], bt)
            nc.scalar.copy(ones_b[:, :], ones_f[:, :])
            masksb = constp.tile([128, KC], f32)
            nc.sync.dma_start(masksb[:, :], maskneg_in[:, :])
            bo_sb = constp.tile([1, F], fr)
            nc.sync.dma_start(bo_sb[:, :], bo_in[:, :])
            bo_rep = constp.tile([128, F], f32)

            wo = [constp.tile([128, F], bt, name=f"wo{i}") for i in range(H // 2)]
            for i in range(H // 2):
                nc.sync.dma_start(wo[i][:, :], wo_in[i * 128:(i + 1) * 128, :])

            # bo broadcast to 128 partitions, once
            for nb2 in range(2):
                ps = psAB.tile([128, 512], f32, tag="psab", name="psab")
                nc.tensor.matmul(ps[:, :], ones_r[0:1, 0:128],
                                 bo_sb[0:1, nb2 * 512:(nb2 + 1) * 512],
                                 start=True, stop=True)
                nc.scalar.copy(bo_rep[:, nb2 * 512:(nb2 + 1) * 512], ps[:, :])

            def body(_iv=None):
                with (
                    tc.tile_pool(name="qTp", bufs=1) as qTp,
                    tc.tile_pool(name="EBp", bufs=1) as EBp,
                    tc.tile_pool(name="oPp", bufs=1) as oPp,
                    tc.tile_pool(name="vSp", bufs=1) as vSp,
                    tc.tile_pool(name="wload", bufs=16) as wlp,
                ):
                    qT = [qTp.tile([128, Q], bt, tag=f"qT{i}", name=f"qT{i}")
                          for i in range(FC)]
                    EB = [EBp.tile([128, Q], bt, tag=f"EB{i}", name=f"EB{i}")
                          for i in range(KC)]
                    oPair = [oPp.tile([128, Q], bt, tag=f"oP{i}", name=f"oP{i}")
                             for i in range(H // 2)]
                    vSB = [vSp.tile([128, H * DV], bt, tag=f"vS{i}",
                                    name=f"vS{i}") for i in range(KC)]

                    cp_flip = [0]

                    def cpy(dst, src, eng=None):
                        if eng is None:
                            eng = cp_flip[0] % 2
                            cp_flip[0] += 1
                        if eng == 0:
                            nc.scalar.copy(dst, src)
                        else:
                            nc.vector.tensor_copy(dst, src)

                    with (
                        tc.tile_pool(name="xTp", bufs=1) as xTp,
                        tc.tile_pool(name="kst", bufs=3) as kstp,
                        tc.tile_pool(name="bn", bufs=4) as bnp,
                    ):
                        xT = [xTp.tile([128, NK], bt, tag=f"xT{i}", name=f"xT{i}")
                              for i in range(FC)]

                        # ---- input DMA issue order: x, xq, bias, wv, wq ----
                        xns_all = []
                        with tc.tile_pool(name="xn", bufs=8) as xnp:
                            for rc in range(16):
                                xn = xnp.tile([128, F], bt, name="xn", tag="xn")
                                nc.sync.dma_start(
                                    xn[:, :], x_in[rc * 128:(rc + 1) * 128, :])
                                xns_all.append(xn)
                            xqs = []
                            for r4 in range(4):
                                xn = xnp.tile([128, F], bt, name="xn", tag="xn")
                                nc.sync.dma_start(
                                    xn[:, :], xq_in[r4 * 128:(r4 + 1) * 128, :])
                                xqs.append(xn)
                            bnsh = []
                            for qc in range(4):
                                bn = bnp.tile([128, F], f32, name="bn", tag="bn")
                                nc.sync.dma_start(
                                    bn[:, :],
                                    bias_in[qc * 128:(qc + 1) * 128, 0:F])
                                bnsh.append(bn)
                            wv = [wlp.tile([128, F], bt, tag="w", name="w")
                                  for _ in range(FC)]
                            for fc in range(FC):
                                nc.sync.dma_start(
                                    wv[fc][:, :],
                                    wkv_in[fc * 128:(fc + 1) * 128, F:2 * F])
                            wq = [wlp.tile([128, F], bt, tag="w", name="w")
                                  for _ in range(FC)]
                            for fc in range(FC):
                                nc.sync.dma_start(
                                    wq[fc][:, :],
                                    wq_in[fc * 128:(fc + 1) * 128, :])

                            # ---- A1: transpose x -> xT, xq -> xqT ----
                            with tc.tile_pool(name="xqTp", bufs=1) as xqTp:
                                xqT = [xqTp.tile([128, Q], bt, tag=f"xqT{i}",
                                                 name=f"xqT{i}")
                                       for i in range(FC)]
                                for rg in range(4):
                                    for fc in range(FC):
                                        ps = psAB.tile([128, 512], bt,
                                                       tag="psab", name="psab_t")
                                        for r4 in range(4):
                                            nc.tensor.transpose(
                                                ps[:, r4 * 128:(r4 + 1) * 128],
                                                xns_all[rg * 4 + r4]
                                                [:, fc * 128:(fc + 1) * 128],
                                                ident_b[:, :])
                                        cpy(xT[fc][:, rg * 512:(rg + 1) * 512],
                                            ps[:, :])
                                for fc in range(FC):
                                    ps = psAB.tile([128, 512], bt,
                                                   tag="psab", name="psab_t")
                                    for r4 in range(4):
                                        nc.tensor.transpose(
                                            ps[:, r4 * 128:(r4 + 1) * 128],
                                            xqs[r4][:, fc * 128:(fc + 1) * 128],
                                            ident_b[:, :])
                                    cpy(xqT[fc][:, :], ps[:, :])

                                # ---- B: EB = exp(bias^T + mask) (ACT),
                                #      hidden under A4/A2 PE work; bias
                                #      loaded in column halves to fit SBUF ----
                                for half in range(2):
                                    if half == 1:
                                        bnsh = []
                                        for qc in range(4):
                                            bn = bnp.tile([128, F], f32,
                                                          name="bn", tag="bn")
                                            nc.sync.dma_start(
                                                bn[:, :],
                                                bias_in[qc * 128:(qc + 1) * 128,
                                                        F:2 * F])
                                            bnsh.append(bn)
                                    for kc in range(half * 8, half * 8 + 8):
                                        kcl = kc - half * 8
                                        ps = psAB.tile([128, 512], f32,
                                                       tag="psab", name="psab")
                                        for qc in range(4):
                                            nc.tensor.transpose(
                                                ps[:, qc * 128:(qc + 1) * 128],
                                                bnsh[qc][:, kcl * 128:
                                                         (kcl + 1) * 128],
                                                ident[:, :])
                                        nc.scalar.activation(
                                            EB[kc][:, :], ps[:, :], AF.Exp,
                                            bias=masksb[:, kc:kc + 1], scale=1.0)

                                # ---- A4: v' = [x @ Wv | 1] -> SBUF ----
                                for kc in range(KC):
                                    for half in range(2):
                                        ps = psAB.tile([128, 512], f32,
                                                       tag="psab", name="psab")
                                        for fc in range(FC):
                                            nc.tensor.matmul(
                                                ps[:, :],
                                                xT[fc][:, kc * 128:(kc + 1) * 128],
                                                wv[fc][:, half * 512:(half + 1) * 512],
                                                start=(fc == 0),
                                                stop=(fc == FC - 1))
                                        dst = vSB[kc][:, half * 8 * DV:
                                                      (half + 1) * 8 * DV] \
                                            .rearrange("p (h x) -> p h x",
                                                       x=DV)[:, :, 0:64]
                                        cpy(dst,
                                            ps[:, :].rearrange(
                                                "p (h d) -> p h d", d=64))
                                    ones_dst = vSB[kc][:, :].rearrange(
                                        "p (h x) -> p h x", x=DV)[:, :, 64:65]
                                    nc.gpsimd.tensor_copy(
                                        ones_dst,
                                        ones_b[:, 0:H].rearrange(
                                            "p (a b) -> p a b", b=1))

                                # ---- A2: qT = (Wq*scale)^T @ xqT ----
                                for m in range(FC):
                                    ps = psAB.tile([128, 512], f32,
                                                   tag="psab", name="psab")
                                    for fc in range(FC):
                                        nc.tensor.matmul(
                                            ps[:, :],
                                            wq[fc][:, m * 128:(m + 1) * 128],
                                            xqT[fc][:, :],
                                            start=(fc == 0), stop=(fc == FC - 1))
                                    cpy(qT[m][:, :], ps[:, :])

                        # wk reuses the wq/wv slots (consumers done)
                        wk = [wlp.tile([128, F], bt, tag="w", name="w")
                              for _ in range(FC)]
                        for fc in range(FC):
                            nc.sync.dma_start(
                                wk[fc][:, :], wkv_in[fc * 128:(fc + 1) * 128, 0:F])

                        if "C" not in STAGES:
                            with tc.tile_pool(name="dbg", bufs=2) as dbgp:
                                dbf = dbgp.tile([128, 512], f32, name="dbf")
                                nc.vector.tensor_copy(dbf[:, :], EB[0][:, :])
                                nc.sync.dma_start(out_t[0:128, 0:512], dbf[:, :])
                            return

                        # ---- C (+A3 interleaved): attention per head pair ----
                        with (
                            tc.tile_pool(name="ef", bufs=3) as efp,
                            tc.tile_pool(name="et", bufs=5) as ep,
                            tc.tile_pool(name="dsb", bufs=2) as dsbp,
                            tc.tile_pool(name="rrep", bufs=2) as rrepp,
                        ):
                            def emit_kT(hp):
                                # A3 slice: kT rows for one head pair (SBUF)
                                kst = kstp.tile([128, NK], bt, name="kst")
                                for nb in range(NB):
                                    ps = psAB.tile([128, 512], f32,
                                                   tag="psab", name="psab")
                                    for fc in range(FC):
                                        nc.tensor.matmul(
                                            ps[:, :],
                                            wk[fc][:, hp * 128:(hp + 1) * 128],
                                            xT[fc][:, nb * 512:(nb + 1) * 512],
                                            start=(fc == 0), stop=(fc == FC - 1))
                                    cpy(kst[:, nb * 512:(nb + 1) * 512],
                                        ps[:, :], eng=1)
                                return kst

                            def emit_tail(psU2o, hpo):
                                # denominators, reciprocal, normalize
                                for sub in range(2):
                                    Dsb = dsbp.tile([DV, 512], fr, name="Dsb")
                                    nc.vector.tensor_copy(Dsb[64:65, :],
                                                          psU2o[sub][64:65, :])
                                    psR = psAB.tile([128, 512], f32,
                                                    tag="psab", name="psab")
                                    nc.tensor.matmul(psR[0:64, :],
                                                     ones_r[64:65, 0:64],
                                                     Dsb[64:65, :],
                                                     start=True, stop=True)
                                    rrep = rrepp.tile([64, 512], f32,
                                                      name="rrep")
                                    nc.vector.reciprocal_approx_fast(
                                        out=rrep[:, :], in_=psR[0:64, :])
                                    nc.vector.tensor_mul(
                                        oPair[hpo][sub * 64:(sub + 1) * 64, :],
                                        psU2o[sub][0:64, :], rrep[:, :])

                            kst_next = emit_kT(0)
                            tail_deferred = [None]
                            for hp in range(H // 2):
                                kst = kst_next

                                psU2 = [psUp.tile([DV, 512], f32, name="psu")
                                        for _ in range(2)]
                                pending = []

                                def drain_av(upto):
                                    while pending and pending[0][0] <= upto:
                                        kc0, eT0 = pending.pop(0)
                                        for sub in range(2):
                                            nc.tensor.matmul(
                                                psU2[sub][:, :],
                                                vSB[kc0][:, (2 * hp + sub) * DV:
                                                         (2 * hp + sub + 1) * DV],
                                                eT0[:, sub * 512:(sub + 1) * 512],
                                                start=(kc0 == 0),
                                                stop=(kc0 == KC - 1))

                                for kc in range(KC):
                                    ps = psCp.tile([128, 1024], f32, name="psc")
                                    for sub in range(2):
                                        po = sub * 64
                                        nc.tensor.matmul(
                                            ps[:, sub * 512:(sub + 1) * 512],
                                            kst[po:po + 64,
                                                kc * 128:(kc + 1) * 128],
                                            qT[hp][po:po + 64, :],
                                            start=True, stop=True)
                                    ef = efp.tile([128, 1024], bt, name="ef")
                                    nc.scalar.activation(
                                        ef[:, :], ps[:, :], AF.Exp,
                                        scale=1.0)
                                    eT = ep.tile([128, 1024], bt, name="eT")
                                    for sub in range(2):
                                        nc.vector.tensor_mul(
                                            eT[:, sub * 512:(sub + 1) * 512],
                                            ef[:, sub * 512:(sub + 1) * 512],
                                            EB[kc][:, :])
                                    pending.append((kc, eT))
                                    drain_av(kc - 4)
                                    if kc == 2 and tail_deferred[0] is not None:
                                        emit_tail(*tail_deferred[0])
                                        tail_deferred[0] = None
                                # pre-produce the next pair's kT so the PE
                                # fills ACT's remaining exp window instead of
                                # stalling behind the tail chain
                                if hp + 1 < H // 2:
                                    kst_next = emit_kT(hp + 1)
                                drain_av(KC)
                                if hp + 1 < H // 2:
                                    tail_deferred[0] = (psU2, hp)
                                else:
                                    emit_tail(psU2, hp)

                    if "D" not in STAGES:
                        with tc.tile_pool(name="dbg", bufs=2) as dbgp:
                            dbf = dbgp.tile([128, 512], f32, name="dbf")
                            nc.vector.tensor_copy(dbf[:, :], oPair[0][:, :])
                            nc.sync.dma_start(out_t[0:128, 0:512], dbf[:, :])
                        return

                    # ======== stage D ========
                    with tc.tile_pool(name="fin", bufs=3) as finp:
                        for mc in range(4):
                            for nb2 in range(2):
                                psF = psAB.tile([128, 512], f32,
                                                tag="psab", name="psab")
                                for hp in range(H // 2):
                                    nc.tensor.matmul(
                                        psF[:, :],
                                        oPair[hp][:, mc * 128:(mc + 1) * 128],
                                        wo[hp][:, nb2 * 512:(nb2 + 1) * 512],
                                        start=(hp == 0),
                                        stop=(hp == H // 2 - 1))
                                fin = finp.tile([128, 512], f32, name="fin")
                                nc.vector.tensor_add(
                                    fin[:, :], psF[:, :],
                                    bo_rep[:, nb2 * 512:(nb2 + 1) * 512])
                                nc.sync.dma_start(
                                    out_t[mc * 128:(mc + 1) * 128,
                                          nb2 * 512:(nb2 + 1) * 512],
                                    fin[:, :])

            if niter == 1:
                body()
            else:
                with tc.For_i(0, niter, 1) as iv:
                    body(iv)

    nc.finalize()
    return nc


_nc_cache = {}


def _get_nc(niter=1):
    if niter not in _nc_cache:
        _nc_cache[niter] = build_nc(niter)
    return _nc_cache[niter]


def make_in_maps(x, bias, mask, Wq, Wkv, Wo, bo):
    bf16 = ml_dtypes.bfloat16
    x = np.asarray(x, dtype=np.float32)
    bias = np.asarray(bias, dtype=np.float32)
    mask = np.asarray(mask)
    x_b = x.astype(bf16)
    wq_b = (np.asarray(Wq, dtype=np.float32) * SCALE).astype(bf16)
    wkv_b = np.asarray(Wkv, dtype=np.float32).astype(bf16)
    wo_b = np.asarray(Wo, dtype=np.float32).astype(bf16)
    in_maps = []
    for c in range(8):
        b, qi = c // 4, c % 4
        q0 = qi * Q
        maskneg = np.where(mask[b], 0.0, MASK_NEG).astype(np.float32)
        in_maps.append({
            "x_in": np.ascontiguousarray(x_b[b]),
            "xq_in": np.ascontiguousarray(x_b[b, q0:q0 + Q]),
            "bias_in": np.ascontiguousarray(bias[b, q0:q0 + Q]),
            "maskneg_in": np.ascontiguousarray(maskneg.reshape(KC, 128).T),
            "wq_in": wq_b,
            "wkv_in": wkv_b,
            "wo_in": wo_b,
            "bo_in": np.ascontiguousarray(
                np.asarray(bo, dtype=np.float32).reshape(1, F)),
        })
    return in_maps


class _CachedRunner:
    """Jit the NEFF-backed executable once; repeat kernel() calls then skip
    the ~40s relower/recompile and run in ~0.1s."""

    def __init__(self, nc, n_cores=8):
        import jax
        from jax.sharding import Mesh, PartitionSpec
        from jax.experimental.shard_map import shard_map
        from concourse.bass2jax import (_bass_exec_p, install_neuronx_cc_hook,
                                        partition_id_tensor)
        install_neuronx_cc_hook()
        self.jax = jax
        self.n_cores = n_cores
        pname = nc.partition_id_tensor.name if nc.partition_id_tensor else None
        in_names, out_names, out_avals, zeros = [], [], [], []
        for alloc in nc.m.functions[0].allocations:
            if not isinstance(alloc, mybir.MemoryLocationSet):
                continue
            name = alloc.memorylocations[0].name
            if alloc.kind == "ExternalInput":
                if name != pname:
                    in_names.append(name)
            elif alloc.kind == "ExternalOutput":
                out_names.append(name)
                shape = tuple(alloc.tensor_shape)
                dt_np = mybir.dt.np(alloc.dtype)
                out_avals.append(jax.core.ShapedArray(shape, dt_np))
                zeros.append(np.zeros(shape, dt_np))
        self.in_names, self.out_names = in_names, out_names
        self.out_avals, self.zeros = out_avals, zeros
        all_names = in_names + out_names + ([pname] if pname else [])

        def _body(*args):
            ops = list(args)
            if pname is not None:
                ops.append(partition_id_tensor())
            return tuple(_bass_exec_p.bind(
                *ops, out_avals=tuple(out_avals), in_names=tuple(all_names),
                out_names=tuple(out_names), lowering_input_output_aliases=(),
                sim_require_finite=True, sim_require_nnan=True, nc=nc))

        mesh = Mesh(np.asarray(jax.devices()[:n_cores]), ("core",))
        spec_in = (PartitionSpec("core"),) * (len(in_names) + len(out_names))
        spec_out = (PartitionSpec("core"),) * len(out_names)
        self.fn = jax.jit(shard_map(_body, mesh=mesh, in_specs=spec_in,
                                    out_specs=spec_out, check_rep=False),
                          keep_unused=True)

    def run(self, in_maps):
        n = self.n_cores
        args = [np.concatenate([np.asarray(in_maps[c][k]) for c in range(n)], axis=0)
                for k in self.in_names]
        args += [np.zeros((n * z.shape[0], *z.shape[1:]), z.dtype)
                 for z in self.zeros]
        outs = self.fn(*args)
        self.jax.block_until_ready(outs)
        return [{k: np.asarray(outs[i]).reshape(n, *self.out_avals[i].shape)[c]
                 for i, k in enumerate(self.out_names)} for c in range(n)]


_runner_cache = {}


def kernel(x, bias, mask, Wq, Wkv, Wo, bo):
    in_maps = make_in_maps(x, bias, mask, Wq, Wkv, Wo, bo)
    try:
        if "r" not in _runner_cache:
            _runner_cache["r"] = _CachedRunner(_get_nc(1))
        results = _runner_cache["r"].run(in_maps)
    except Exception:
        _runner_cache.pop("r", None)
        res = run_bass_kernel_spmd(_get_nc(1), in_maps, core_ids=list(range(8)))
        results = res.results
    out = np.empty((2, NK, F), dtype=np.float32)
    for c in range(8):
        b, qi = c // 4, c % 4
        out[b, qi * Q:(qi + 1) * Q] = results[c]["out_t"]
    return out
